# revision 38
# baseline (speedup 1.0000x reference)
"""CPAMDec attention-decoder kernel for 8 Trainium2 NeuronCores.

Reference computation (per batch n of N=8):
    q  = x_n^T @ wq^T + bq          (HW=4096, C4=128)
    k  = y_n @ wk^T + bk            (K=32, C4=128)
    v  = y_n @ wv^T + bv            (K=32, C=512)
    attn = softmax(q @ k^T, axis=-1)        (HW, K)
    out = scale * (v^T @ attn^T) + x_n      (C, HW)

Sharding: pure data parallel - core i computes batch i.

The device computes only the attention delta  d = scale*(v^T@attn^T)
and ships it back as int8 with a per-partition dequant scale; the host
adds the residual x (which it already holds in fp32).  All K=32-sized
projection prep (k', the fused energy weights M = wq@k'^T, the exp
bias e_b, the stacked value matrix) is host-precomputed into one small
packed-constant tile - these are <1% of the model FLOPs; the device
keeps all O(HW) work: the 4096-pixel energy matmuls, softmax, and the
out-projection.

Why int8: attn is a convex combination, so |d[c,p]| <= max_j
|scale*(v[j,c]+bv[c])|, a bound computed exactly from the SAME fp16
values the device multiplies with.  Quantizing d to int8 against that
per-channel bound keeps end-to-end rel-err ~2.3e-3 (gate is 2e-2)
while halving store bytes vs fp16.  Per-core HBM traffic: x16 4.19MB
+ pk 0.16MB + delta 2.10MB = 6.45MB.

Per-chunk engine schedule (chunk = 512 pixels, cadence ~2.1us; ACT/DVE
saturated, PE cold-clock-bound early, DMA co-limited):
  PE:   4 energy MMs (fused M weights, reads x directly),
        1 key-sum MM, 4 row-tiled out MMs (concurrent)        ~1.5us
  ACT:  exp (bias=e_b per-partition, exact bq folding), evac half A
        (Copy with per-partition int8 scale)                  ~1.9us
  DVE:  reciprocal_approx_fast, evac half B (tensor_scalar)   ~2.0us
  Pool: attn = expt * rec (mixed fp16*fp32)                   ~1.2us
  Sync: 2 half-store triggers                                 ~1.2us

HW-measured notes: the HAM clock gate on this part ignores kernel
activity (a free-running SW window opens ~15-20us in), so warm-up
matmuls are pure waste; pk loads FIRST on the sync ring (FIFO within
a logical DMA queue beats round-robin with the x stream).
"""

import sys

sys.path.insert(0, "/opt/trn_rl_repo")

import numpy as np

import concourse.bacc as bacc
import concourse.mybir as mybir
import concourse.tile as tile
from concourse.bass_utils import run_bass_kernel_spmd

F32 = mybir.dt.float32
F16 = mybir.dt.float16
I8 = mybir.dt.int8
AF = mybir.ActivationFunctionType
ALU = mybir.AluOpType

N, C, H, W, K = 8, 512, 64, 64, 32
HW = H * W            # 4096
C4 = C // 4           # 128
PC = 512              # free-dim chunk (1 PSUM bank of fp32)
NPC = HW // PC        # 8 chunks
KC = C // 128         # 4 contraction chunks
CT = C // 128         # 4 output row-tiles
CW = KC * PC          # 2048 elements per chunk per partition
PKW = 4 * 128 + 128 + 2   # packed consts: M(4 tiles)|vstack|e_b|dsc
ESHIFT = -6.0         # exp shift: keeps exp() outputs in fp16 range
DMARGIN = 1.08        # int8 bound safety margin


def _emit(nc, tc):
    sync = nc.sync

    with (
        tc.tile_pool(name="const", bufs=1) as cst,
        tc.tile_pool(name="xbuf", bufs=1) as xp,
        tc.tile_pool(name="work", bufs=3) as wk_pool,
        tc.tile_pool(name="ps", bufs=2, space="PSUM") as ps,
    ):
        # memset-backed constant first on the gpsimd queue
        ones32 = cst.tile([K, 128], F16, name="ones32", tag="ones32")
        nc.gpsimd.memset(ones32[:], 1.0)

        # ---- pk gates the whole pipeline, so it goes FIRST on the
        # sync ring: one logical DMA queue drains FIFO, so pk lands
        # before any x data instead of round-robining with it.
        pk = cst.tile([128, PKW], F16, name="pk", tag="pk")
        sync.dma_start(pk[:], nc.t.pk[:])

        def m_t(k):
            return pk[:, k * 128:(k + 1) * 128]

        vstack = pk[:, 512:640]             # [128, 128] stacked value
        eb_col = pk[:, 640:641]             # [128, 1] exp bias (fp16)
        dsc_col = pk[:, 641:642]            # [128, 1] int8 scale (fp16)

        # ---- x prefetch on the sync ring behind pk (store triggers
        # queue behind them). Chunks 0,1 singly; the rest in 1MB pairs.
        xs = [None] * NPC
        # chunk 0 loads as two half-DMAs so energy(0)'s first matmuls
        # can start as soon as the first half lands.
        x0 = xp.tile([128, CW], F16, name="xs0", tag="xs0")
        sync.dma_start(x0[:, 0:CW // 2], nc.t.x16[:, 0:CW // 2])
        sync.dma_start(x0[:, CW // 2:CW], nc.t.x16[:, CW // 2:CW])
        xs[0] = x0
        for pc in (1,):
            t = xp.tile([128, CW], F16, name=f"xs{pc}", tag=f"xs{pc}")
            sync.dma_start(t[:], nc.t.x16[:, pc * CW:(pc + 1) * CW])
            xs[pc] = t
        for pc in (2, 4, 6):
            t = xp.tile([128, 2 * CW], F16, name=f"xs{pc}", tag=f"xs{pc}")
            sync.dma_start(t[:], nc.t.x16[:, pc * CW:(pc + 2) * CW])
            xs[pc] = t[:, 0:CW]
            xs[pc + 1] = t[:, CW:2 * CW]

        # fp16 -> fp32 for the ACT/DVE scale and bias operands
        dsc32 = cst.tile([128, 1], F32, name="dsc32", tag="dsc32")
        nc.scalar.activation(out=dsc32[:], in_=dsc_col, func=AF.Copy,
                             scale=1.0)
        eb32 = cst.tile([128, 1], F32, name="eb32", tag="eb32")
        nc.scalar.activation(out=eb32[:], in_=eb_col, func=AF.Copy,
                             scale=1.0)

        # ------------- software-pipelined main loop over column chunks
        #   step:   energy/exp(step)   sum/rec/mul(step-1)
        #           out/evac/store(step-2)
        expts = [None] * NPC
        attns = [None] * NPC

        def stage_energy(pc):
            # fused q+energy: e = M^T x straight from the x chunk
            xt = xs[pc]
            e_ps = ps.tile([128, PC], F32, name=f"e_ps{pc}", tag="e", bufs=2)
            for k in range(KC):
                nc.tensor.matmul(e_ps[:], m_t(k),
                                 xt[:, k * PC:(k + 1) * PC],
                                 start=(k == 0), stop=(k == KC - 1))
            expt = wk_pool.tile([128, PC], F16, name="expt", tag="expt",
                                bufs=4)
            nc.scalar.activation(out=expt[:], in_=e_ps[:], func=AF.Exp,
                                 bias=eb32[:], scale=1.0)
            expts[pc] = expt

        def stage_softmax(pc):
            s_ps = ps.tile([128, PC], F32, name=f"s_ps{pc}", tag="s", bufs=2)
            nc.tensor.matmul(s_ps[:], ones32[:], expts[pc][0:K, :],
                             start=True, stop=True)
            rec = wk_pool.tile([128, PC], F32, name="rec", tag="rec", bufs=4)
            nc.vector.reciprocal_approx_fast(out=rec[:], in_=s_ps[:])
            # Pool does the normalize (mixed fp16*fp32 -> fp16); keeps
            # ACT/DVE free for the evac halves.
            attn = wk_pool.tile([128, PC], F16, name="attn", tag="attn",
                                bufs=4)
            nc.gpsimd.tensor_mul(attn[:], expts[pc][:], rec[:])
            attns[pc] = attn

        def stage_out(pc):
            attn = attns[pc]
            # two independent [128, 2*PC] PSUM halves (2 banks each):
            # the A and B halves pipeline independently across chunks
            # (MM-A(c+1) only waits on evacA(c), not evacB(c)).
            oA = ps.tile([128, 2 * PC], F32, name=f"oA{pc}", tag="oA",
                         bufs=1)
            oB = ps.tile([128, 2 * PC], F32, name=f"oB{pc}", tag="oB",
                         bufs=1)
            for ct in range(CT):
                tgt = oA if ct < 2 else oB
                col = (ct % 2) * PC
                nc.tensor.matmul(tgt[:, col:col + PC],
                                 vstack[32 * ct:32 * (ct + 1), :],
                                 attn[32 * ct:32 * (ct + 1), :],
                                 start=True, stop=True,
                                 tile_position=(32 * ct, 0))
            osb = wk_pool.tile([128, CT * PC], I8, name="osb", tag="osb",
                               bufs=3)
            nc.scalar.activation(out=osb[:, 0:2 * PC], in_=oA[:],
                                 func=AF.Copy, scale=dsc32[:])
            nc.vector.tensor_scalar(out=osb[:, 2 * PC:4 * PC], in0=oB[:],
                                    scalar1=dsc32[:], scalar2=None,
                                    op0=ALU.mult)
            # two half-stores: the ACT half doesn't wait on the DVE half
            sync.dma_start(nc.t.oq8[:, pc * CW:pc * CW + 2 * PC],
                           osb[:, 0:2 * PC])
            sync.dma_start(nc.t.oq8[:, pc * CW + 2 * PC:(pc + 1) * CW],
                           osb[:, 2 * PC:4 * PC])

        # softmax(0) is emitted right after energy(0) (one-time ramp
        # shortcut: sum(0) would otherwise queue behind energy(1) in
        # the PE FIFO); out(s-2) leads each later step so the first
        # stores fire as early as possible.
        stage_energy(0)
        stage_softmax(0)
        for step in range(1, NPC + 2):
            if 0 <= step - 2 < NPC:
                stage_out(step - 2)
            if step < NPC:
                stage_energy(step)
            if 1 <= step - 1 < NPC:
                stage_softmax(step - 1)


class _T:
    """Attribute access to declared dram params."""
    def __init__(self):
        self.__dict__ = {}


_NC_CACHE = []


def _build():
    if _NC_CACHE:
        return _NC_CACHE[0]
    nc = bacc.Bacc(target_bir_lowering=False)
    nc.t = _T()
    t = nc.t
    t.x16 = nc.declare_dram_parameter("x16", [128, NPC * CW], F16,
                                      isOutput=False)
    t.pk = nc.declare_dram_parameter("pk", [128, PKW], F16,
                                     isOutput=False)
    t.oq8 = nc.declare_dram_parameter("oq8", [128, NPC * CW], I8,
                                      isOutput=True)
    with tile.TileContext(nc) as tc:
        _emit(nc, tc)
    nc.finalize()
    _NC_CACHE.append(nc)
    return nc


def _prep(x, y, wq, bq, wk, bk, wv, bv, scale):
    """Host-side input packing; returns (in_maps, inv_dsc[N,128])."""
    f16, f32 = np.float16, np.float32
    # x: (N,C,H,W) -> per-core [128, NPC*KC*PC] partition-major fp16,
    # so every chunk DMA line is 4KB contiguous per partition.
    x16 = (np.asarray(x, dtype=f32)
           .reshape(N, KC, 128, NPC, PC)
           .transpose(0, 2, 3, 1, 4)
           .reshape(N, 128, NPC * CW)
           .astype(f16))

    wq32, wk32, wv32, y32 = (np.asarray(a, f32) for a in (wq, wk, wv, y))
    s0 = f32(np.asarray(scale, f32).reshape(-1)[0])

    # K=32-sized projection prep (mirrors what the device prologue
    # used to compute, at slightly better fp32 precision):
    #   k' (with bk), fused energy weights M, exp bias e_b, and the
    #   scale-folded stacked value matrix.
    k16 = (y32 @ wk32.T + bk).astype(f16).astype(f32)       # [N, K, C4]
    M = np.einsum('oc,njo->ncj', wq32, k16)                  # [N, C, K]
    Mrep = np.tile(M, (1, 1, 4))                             # [N, C, 4K]
    # pk slice k holds [c' = c-128k rows, 4K cols]
    pk_M = Mrep.reshape(N, KC, 128, 4 * K).transpose(0, 2, 1, 3) \
               .reshape(N, 128, KC * 4 * K)
    eb = np.einsum('njo,o->nj', k16, np.asarray(bq, f32)) + ESHIFT
    eb_rep = np.tile(eb, (1, 4)).reshape(N, 128, 1)          # [N, 4K, 1]

    vsb = (s0 * (y32 @ wv32.T + bv)).astype(f16)             # [N, K, C]
    # vstack[32*ct + j, m] = vsb[j, 128*ct + m]
    vs = (vsb.astype(f32).reshape(N, K, CT, 128)
          .transpose(0, 2, 1, 3).reshape(N, 128, 128))

    # per-partition int8 scale: |delta[c,p]| <= max_j |vsb[j,c]| (attn
    # is convex); partition m serves channels {m,128+m,256+m,384+m}.
    dmax = np.abs(vs).max(axis=1)                            # [N, 128]
    dsc16 = (127.0 / (dmax * DMARGIN + 1e-30)).astype(f16)   # [N, 128]
    inv_dsc = 1.0 / dsc16.astype(f32)                        # exact inverse

    pk_n = [
        np.concatenate([pk_M[i], vs[i], eb_rep[i],
                        np.float32(dsc16[i]).reshape(128, 1)],
                       axis=1).astype(f16)
        for i in range(N)
    ]
    in_maps = [
        {"x16": np.ascontiguousarray(x16[i]), "pk": pk_n[i]}
        for i in range(N)
    ]
    return in_maps, inv_dsc


def _run(inputs, **kwargs):
    nc = _build()
    in_maps, inv_dsc = _prep(**inputs)
    res = run_bass_kernel_spmd(nc, in_maps,
                               core_ids=list(range(N)), **kwargs)
    res.inv_dsc = inv_dsc
    return res


def kernel(**inputs) -> np.ndarray:
    res = _run(inputs)
    x = np.asarray(inputs["x"], dtype=np.float32)
    # oq8 [128, NPC*CT*PC] int8 partition-major -> delta (C, HW) fp32,
    # dequant by the per-partition scale, then the residual add.
    out = np.empty((N, C, HW), dtype=np.float32)
    for i in range(N):
        d = (res.results[i]["oq8"].astype(np.float32)
             * res.inv_dsc[i][:, None])
        out[i] = (d.reshape(128, NPC, CT, PC)
                  .transpose(2, 0, 1, 3)
                  .reshape(C, HW))
    return out.reshape(N, C, H, W) + x.reshape(N, C, H, W)


# revision 40
# speedup vs baseline: 1.1410x; 1.1410x over previous
"""CPAMDec attention-decoder kernel for 8 Trainium2 NeuronCores.

Reference computation (per batch n of N=8):
    q  = x_n^T @ wq^T + bq          (HW=4096, C4=128)
    k  = y_n @ wk^T + bk            (K=32, C4=128)
    v  = y_n @ wv^T + bv            (K=32, C=512)
    attn = softmax(q @ k^T, axis=-1)        (HW, K)
    out = scale * (v^T @ attn^T) + x_n      (C, HW)

Sharding: pure data parallel - core i computes batch i.

The device computes only the attention delta  d = scale*(v^T@attn^T)
and ships it back as int8 with a per-partition dequant scale; the host
adds the residual x (which it already holds in fp32).  All K=32-sized
projection prep (k', the fused energy weights M = wq@k'^T, the exp
bias e_b, the stacked value matrix) is host-precomputed into one small
packed-constant tile - these are <1% of the model FLOPs; the device
keeps all O(HW) work: the 4096-pixel energy matmuls, softmax, and the
out-projection.

Why int8: attn is a convex combination, so |d[c,p]| <= max_j
|scale*(v[j,c]+bv[c])|, a bound computed exactly from the SAME fp16
values the device multiplies with.  Quantizing d to int8 against that
per-channel bound keeps end-to-end rel-err ~2.3e-3 (gate is 2e-2)
while halving store bytes vs fp16.  Per-core HBM traffic: x16 4.19MB
+ pk 0.16MB + delta 2.10MB = 6.45MB.

Per-chunk engine schedule (chunk = 512 pixels, cadence ~2.1us; ACT/DVE
saturated, PE cold-clock-bound early, DMA co-limited):
  PE:   4 energy MMs (fused M weights, reads x directly),
        1 key-sum MM, 4 row-tiled out MMs (concurrent)        ~1.5us
  ACT:  exp (bias=e_b per-partition, exact bq folding), evac half A
        (Copy with per-partition int8 scale)                  ~1.9us
  DVE:  reciprocal_approx_fast, evac half B (tensor_scalar)   ~2.0us
  Pool: attn = expt * rec (mixed fp16*fp32)                   ~1.2us
  Sync: 2 half-store triggers                                 ~1.2us

HW-measured notes: the HAM clock gate on this part ignores kernel
activity (a free-running SW window opens ~15-20us in), so warm-up
matmuls are pure waste; pk loads FIRST on the sync ring (FIFO within
a logical DMA queue beats round-robin with the x stream).
"""

import sys

sys.path.insert(0, "/opt/trn_rl_repo")

import numpy as np

import concourse.bacc as bacc
import concourse.mybir as mybir
import concourse.tile as tile
from concourse.bass_utils import run_bass_kernel_spmd

F32 = mybir.dt.float32
F16 = mybir.dt.float16
I8 = mybir.dt.int8
AF = mybir.ActivationFunctionType
ALU = mybir.AluOpType

N, C, H, W, K = 8, 512, 64, 64, 32
HW = H * W            # 4096
C4 = C // 4           # 128
PC = 512              # free-dim chunk (1 PSUM bank of fp32)
NPC = HW // PC        # 8 chunks
KC = C // 128         # 4 contraction chunks
CT = C // 128         # 4 output row-tiles
CW = KC * PC          # 2048 elements per chunk per partition
PKW = 4 * 128 + 128 + 2   # packed consts: M(4 tiles)|vstack|e_b|dsc
ESHIFT = -6.0         # exp shift: keeps exp() outputs in fp16 range
DMARGIN = 1.08        # int8 bound safety margin


def _emit(nc, tc):
    sync = nc.sync

    with (
        tc.tile_pool(name="const", bufs=1) as cst,
        tc.tile_pool(name="xbuf", bufs=1) as xp,
        tc.tile_pool(name="work", bufs=3) as wk_pool,
        tc.tile_pool(name="ps", bufs=2, space="PSUM") as ps,
    ):
        # memset-backed constant first on the gpsimd queue
        ones32 = cst.tile([K, 128], F16, name="ones32", tag="ones32")
        nc.gpsimd.memset(ones32[:], 1.0)

        # ---- pk gates the whole pipeline, so it goes FIRST on the
        # sync ring: one logical DMA queue drains FIFO, so pk lands
        # before any x data instead of round-robining with it.
        pk = cst.tile([128, PKW], F16, name="pk", tag="pk")
        sync.dma_start(pk[:], nc.t.pk[:])

        def m_t(k):
            return pk[:, k * 128:(k + 1) * 128]

        vstack = pk[:, 512:640]             # [128, 128] stacked value
        eb_col = pk[:, 640:641]             # [128, 1] exp bias (fp16)
        dsc_col = pk[:, 641:642]            # [128, 1] int8 scale (fp16)

        # ---- x prefetch on the sync ring behind pk (store triggers
        # queue behind them). Chunks 0,1 singly; the rest in 1MB pairs.
        xs = [None] * NPC
        for pc in (0, 1):
            t = xp.tile([128, CW], F16, name=f"xs{pc}", tag=f"xs{pc}")
            sync.dma_start(t[:], nc.t.x16[:, pc * CW:(pc + 1) * CW])
            xs[pc] = t
        for pc in (2, 4, 6):
            t = xp.tile([128, 2 * CW], F16, name=f"xs{pc}", tag=f"xs{pc}")
            sync.dma_start(t[:], nc.t.x16[:, pc * CW:(pc + 2) * CW])
            xs[pc] = t[:, 0:CW]
            xs[pc + 1] = t[:, CW:2 * CW]

        # fp16 -> fp32 for the ACT/DVE scale and bias operands
        dsc32 = cst.tile([128, 1], F32, name="dsc32", tag="dsc32")
        nc.scalar.activation(out=dsc32[:], in_=dsc_col, func=AF.Copy,
                             scale=1.0)
        eb32 = cst.tile([128, 1], F32, name="eb32", tag="eb32")
        nc.scalar.activation(out=eb32[:], in_=eb_col, func=AF.Copy,
                             scale=1.0)

        # ------------- software-pipelined main loop over column chunks
        #   step:   energy/exp(step)   sum/rec/mul(step-1)
        #           out/evac/store(step-2)
        expts = [None] * NPC
        attns = [None] * NPC

        def stage_energy(pc):
            # fused q+energy: e = M^T x straight from the x chunk
            xt = xs[pc]
            e_ps = ps.tile([128, PC], F32, name=f"e_ps{pc}", tag="e", bufs=2)
            for k in range(KC):
                nc.tensor.matmul(e_ps[:], m_t(k),
                                 xt[:, k * PC:(k + 1) * PC],
                                 start=(k == 0), stop=(k == KC - 1))
            expt = wk_pool.tile([128, PC], F16, name="expt", tag="expt",
                                bufs=4)
            nc.scalar.activation(out=expt[:], in_=e_ps[:], func=AF.Exp,
                                 bias=eb32[:], scale=1.0)
            expts[pc] = expt

        def stage_softmax(pc):
            s_ps = ps.tile([128, PC], F32, name=f"s_ps{pc}", tag="s", bufs=2)
            nc.tensor.matmul(s_ps[:], ones32[:], expts[pc][0:K, :],
                             start=True, stop=True)
            rec = wk_pool.tile([128, PC], F32, name="rec", tag="rec", bufs=4)
            nc.vector.reciprocal_approx_fast(out=rec[:], in_=s_ps[:])
            # Pool does the normalize (mixed fp16*fp32 -> fp16); keeps
            # ACT/DVE free for the evac halves.
            attn = wk_pool.tile([128, PC], F16, name="attn", tag="attn",
                                bufs=4)
            nc.gpsimd.tensor_mul(attn[:], expts[pc][:], rec[:])
            attns[pc] = attn

        def stage_out(pc):
            attn = attns[pc]
            # two independent [128, 2*PC] PSUM halves (2 banks each):
            # the A and B halves pipeline independently across chunks
            # (MM-A(c+1) only waits on evacA(c), not evacB(c)).
            oA = ps.tile([128, 2 * PC], F32, name=f"oA{pc}", tag="oA",
                         bufs=1)
            oB = ps.tile([128, 2 * PC], F32, name=f"oB{pc}", tag="oB",
                         bufs=1)
            for ct in range(CT):
                tgt = oA if ct < 2 else oB
                col = (ct % 2) * PC
                nc.tensor.matmul(tgt[:, col:col + PC],
                                 vstack[32 * ct:32 * (ct + 1), :],
                                 attn[32 * ct:32 * (ct + 1), :],
                                 start=True, stop=True,
                                 tile_position=(32 * ct, 0))
            osb = wk_pool.tile([128, CT * PC], I8, name="osb", tag="osb",
                               bufs=3)
            nc.scalar.activation(out=osb[:, 0:2 * PC], in_=oA[:],
                                 func=AF.Copy, scale=dsc32[:])
            nc.vector.tensor_scalar(out=osb[:, 2 * PC:4 * PC], in0=oB[:],
                                    scalar1=dsc32[:], scalar2=None,
                                    op0=ALU.mult)
            # two half-stores: the ACT half doesn't wait on the DVE half
            sync.dma_start(nc.t.oq8[:, pc * CW:pc * CW + 2 * PC],
                           osb[:, 0:2 * PC])
            sync.dma_start(nc.t.oq8[:, pc * CW + 2 * PC:(pc + 1) * CW],
                           osb[:, 2 * PC:4 * PC])

        # softmax(0) is emitted right after energy(0) (one-time ramp
        # shortcut: sum(0) would otherwise queue behind energy(1) in
        # the PE FIFO); out(s-2) leads each later step so the first
        # stores fire as early as possible.
        stage_energy(0)
        for step in range(1, NPC + 2):
            if step < NPC:
                stage_energy(step)
            if 0 <= step - 1 < NPC:
                stage_softmax(step - 1)
            if 0 <= step - 2 < NPC:
                stage_out(step - 2)


class _T:
    """Attribute access to declared dram params."""
    def __init__(self):
        self.__dict__ = {}


_NC_CACHE = []


def _build():
    if _NC_CACHE:
        return _NC_CACHE[0]
    nc = bacc.Bacc(target_bir_lowering=False)
    nc.t = _T()
    t = nc.t
    t.x16 = nc.declare_dram_parameter("x16", [128, NPC * CW], F16,
                                      isOutput=False)
    t.pk = nc.declare_dram_parameter("pk", [128, PKW], F16,
                                     isOutput=False)
    t.oq8 = nc.declare_dram_parameter("oq8", [128, NPC * CW], I8,
                                      isOutput=True)
    with tile.TileContext(nc) as tc:
        _emit(nc, tc)
    nc.finalize()
    _NC_CACHE.append(nc)
    return nc


def _prep(x, y, wq, bq, wk, bk, wv, bv, scale):
    """Host-side input packing; returns (in_maps, inv_dsc[N,128])."""
    f16, f32 = np.float16, np.float32
    # x: (N,C,H,W) -> per-core [128, NPC*KC*PC] partition-major fp16,
    # so every chunk DMA line is 4KB contiguous per partition.
    x16 = (np.asarray(x, dtype=f32)
           .reshape(N, KC, 128, NPC, PC)
           .transpose(0, 2, 3, 1, 4)
           .reshape(N, 128, NPC * CW)
           .astype(f16))

    wq32, wk32, wv32, y32 = (np.asarray(a, f32) for a in (wq, wk, wv, y))
    s0 = f32(np.asarray(scale, f32).reshape(-1)[0])

    # K=32-sized projection prep (mirrors what the device prologue
    # used to compute, at slightly better fp32 precision):
    #   k' (with bk), fused energy weights M, exp bias e_b, and the
    #   scale-folded stacked value matrix.
    k16 = (y32 @ wk32.T + bk).astype(f16).astype(f32)       # [N, K, C4]
    M = np.einsum('oc,njo->ncj', wq32, k16)                  # [N, C, K]
    Mrep = np.tile(M, (1, 1, 4))                             # [N, C, 4K]
    # pk slice k holds [c' = c-128k rows, 4K cols]
    pk_M = Mrep.reshape(N, KC, 128, 4 * K).transpose(0, 2, 1, 3) \
               .reshape(N, 128, KC * 4 * K)
    eb = np.einsum('njo,o->nj', k16, np.asarray(bq, f32)) + ESHIFT
    eb_rep = np.tile(eb, (1, 4)).reshape(N, 128, 1)          # [N, 4K, 1]

    vsb = (s0 * (y32 @ wv32.T + bv)).astype(f16)             # [N, K, C]
    # vstack[32*ct + j, m] = vsb[j, 128*ct + m]
    vs = (vsb.astype(f32).reshape(N, K, CT, 128)
          .transpose(0, 2, 1, 3).reshape(N, 128, 128))

    # per-partition int8 scale: |delta[c,p]| <= max_j |vsb[j,c]| (attn
    # is convex); partition m serves channels {m,128+m,256+m,384+m}.
    dmax = np.abs(vs).max(axis=1)                            # [N, 128]
    dsc16 = (127.0 / (dmax * DMARGIN + 1e-30)).astype(f16)   # [N, 128]
    inv_dsc = 1.0 / dsc16.astype(f32)                        # exact inverse

    pk_n = [
        np.concatenate([pk_M[i], vs[i], eb_rep[i],
                        np.float32(dsc16[i]).reshape(128, 1)],
                       axis=1).astype(f16)
        for i in range(N)
    ]
    in_maps = [
        {"x16": np.ascontiguousarray(x16[i]), "pk": pk_n[i]}
        for i in range(N)
    ]
    return in_maps, inv_dsc


def _run(inputs, **kwargs):
    nc = _build()
    in_maps, inv_dsc = _prep(**inputs)
    res = run_bass_kernel_spmd(nc, in_maps,
                               core_ids=list(range(N)), **kwargs)
    res.inv_dsc = inv_dsc
    return res


def kernel(**inputs) -> np.ndarray:
    res = _run(inputs)
    x = np.asarray(inputs["x"], dtype=np.float32)
    # oq8 [128, NPC*CT*PC] int8 partition-major -> delta (C, HW) fp32,
    # dequant by the per-partition scale, then the residual add.
    out = np.empty((N, C, HW), dtype=np.float32)
    for i in range(N):
        d = (res.results[i]["oq8"].astype(np.float32)
             * res.inv_dsc[i][:, None])
        out[i] = (d.reshape(128, NPC, CT, PC)
                  .transpose(2, 0, 1, 3)
                  .reshape(C, HW))
    return out.reshape(N, C, H, W) + x.reshape(N, C, H, W)
